# revision 8
# baseline (speedup 1.0000x reference)
"""GAT (graph attention) layer on 8 Trainium2 NeuronCores.

Reference computation (N=8192, F_IN=256, F_OUT=64, alpha=0.2):
    Wh     = h @ W                                  [N, 64]
    f_src  = Wh @ a[:64, 0]                         [N]
    f_dst  = Wh @ a[64:, 0]                         [N]
    e      = leaky_relu(f_src[:,None] + f_dst[None,:], 0.2)
    att    = softmax(where(adj > 0, e, -9e15), axis=1)
    out    = elu(att @ Wh)

Sharding: row-shard N across 8 cores (1024 query rows per core).  During
host-side sharding each core's adj row-block is staged TRANSPOSED and
re-encoded as a bf16 0/1 mask (values preserved exactly), and h is
staged as hT = h.T in bf16 — layout/precision staging only; all
arithmetic (Wh, attention logits, softmax, aggregation, elu) runs on
device.

Algebraic structure (softmax factorization identical to the proven f32
version):
 1. exp(lrelu(u)) = exp(.2 f_src_i) * exp(.2 f_dst_j) * exp(.8 relu(u));
    the first factor cancels in softmax, b_j = exp(.2 f_dst_j) is folded
    into the stationary operand rhs_aug[j,:] = b_j * [Wh_j | 1] whose
    ones-column also yields the softmax denominator Z_i.
 2. exp(.8 relu(u)) = max(exp(.8 u), 1).
 3. The attention matrix is generated TRANSPOSED, pT[j,i]: f_dst_j is a
    per-partition bias, f_src_i a broadcast row.  Two window flavors:
      ACT window:  X = ACT.Exp(fs_row + fd_bias); Xm = DVE.max(X, 1)
      FAC window:  Xm = DVE.(es_row * ed_scalar) max 1   (one 2-op
                   tensor_scalar; es = exp(.8 f_src), ed = exp(.8 f_dst))
    then p = Xm * adj_mask (bf16, DVE 2x mode), and
    accT[f,i] += rhs_aug[j,f]^T pT[j,i] on the PE (bf16, f32 PSUM).
 4. A subset of mask-mults runs on GPSIMD; their (slow) products would
    gate the in-order PE accumulation stream, so those windows'
    matmuls are DEFERRED to the end of the accumulation group while the
    products wait in a deep SBUF ring — GPSIMD streams concurrently
    without ever stalling the PE.
"""

import sys

sys.path.insert(0, "/opt/trn_rl_repo")

import numpy as np

import concourse.bass as bass  # noqa: F401
import concourse.mybir as mybir
import concourse.tile as tile
from concourse import bacc
from concourse.bass_utils import run_bass_kernel_spmd
from concourse.masks import make_identity

N = 8192
F_IN = 256
F_OUT = 64
N_CORES = 8
ROWS = N // N_CORES  # 1024 query rows per core
KC = F_IN // 128  # 2 contraction chunks
MCH = N // 128  # 64 j-chunks
LCH = ROWS // 128  # 8 local row chunks
HG = 4  # j-chunks per pre-phase half-group (whH psum fits half a bank)
NHG = MCH // HG  # 16 half-groups

F32 = mybir.dt.float32
BF16 = mybir.dt.bfloat16
Act = mybir.ActivationFunctionType
Alu = mybir.AluOpType

_CACHE = {}


def _build_nc(repeat=1):
    nc = bacc.Bacc(
        "TRN2",
        target_bir_lowering=False,
        debug=False,
        enable_asserts=False,
        num_devices=N_CORES,
    )

    hT = nc.dram_tensor("hT", [F_IN, N], BF16, kind="ExternalInput")
    hTs = nc.dram_tensor("hTs", [F_IN, ROWS], BF16, kind="ExternalInput")
    adjT = nc.dram_tensor("adjT", [N, ROWS], BF16, kind="ExternalInput")
    W = nc.dram_tensor("W", [F_IN, F_OUT], F32, kind="ExternalInput")
    a = nc.dram_tensor("a", [2 * F_OUT, 1], F32, kind="ExternalInput")
    out = nc.dram_tensor("out", [ROWS, F_OUT], F32, kind="ExternalOutput")

    # DRAM bounce buffer for the f_src broadcast row
    fsd = nc.dram_tensor("fsd", [ROWS], BF16)

    from contextlib import nullcontext

    with tile.TileContext(nc) as tc:
        rep_ctx = tc.For_i(0, repeat, 1) if repeat > 1 else nullcontext()
        with rep_ctx:
            _kernel_body(nc, tc, hT, hTs, adjT, W, a, out, fsd)

    nc.compile()
    return nc


def _kernel_body(nc, tc, hT, hTs, adjT, W, a, out, fsd):
    import os

    ADJBUFS = int(os.environ.get("GAT_ADJBUFS", "24"))
    FACWIN = int(os.environ.get("GAT_FACWIN", "20"))  # factorized windows
    POOLM = int(os.environ.get("GAT_POOLM", "28"))  # deferred gpsimd mults

    def spread(k):
        if k <= 0:
            return set()
        stride = MCH / k
        return {int(stride * i + stride / 2) for i in range(k)}

    facset = spread(FACWIN)
    poolset = spread(POOLM)
    poolset.discard(0)  # window 0 carries the accumulation start flag
    mm_tail = sorted(poolset)

    with (
        tc.tile_pool(name="consts", bufs=1) as consts,
        tc.tile_pool(name="hp", bufs=3) as hp,
        tc.tile_pool(name="adjp", bufs=ADJBUFS) as adjp,
        tc.tile_pool(name="xk", bufs=3) as xk,
        tc.tile_pool(name="mk", bufs=4) as mk,
        tc.tile_pool(name="pk", bufs=4) as pk,
        tc.tile_pool(name="pdef", bufs=max(len(mm_tail), 1)) as pdef,
        tc.tile_pool(name="ep", bufs=1) as ep,
        tc.tile_pool(name="psW", bufs=3, space="PSUM") as psW,
        tc.tile_pool(name="psM", bufs=2, space="PSUM") as psM,
        tc.tile_pool(name="psAcc", bufs=1, space="PSUM") as psAcc,
    ):
        # ---------------- W-side constants (tiny, f32) ----------------
        idf = consts.tile([128, 128], F32)
        make_identity(nc, idf)

        a2 = consts.tile([64, 2], F32)
        nc.gpsimd.dma_start(out=a2[:, 0:1], in_=a[0:F_OUT, :])
        nc.gpsimd.dma_start(out=a2[:, 1:2], in_=a[F_OUT : 2 * F_OUT, :])
        a2s = consts.tile([64, 2], F32)
        nc.vector.tensor_scalar(a2s[:, 0:1], a2[:, 0:1], 0.8, None, Alu.mult)
        nc.vector.tensor_copy(a2s[:, 1:2], a2[:, 1:2])

        Wf = consts.tile([128, KC, F_OUT + 1], F32)
        nc.gpsimd.dma_start(
            out=Wf[:, :, 0:F_OUT],
            in_=W[:, :].rearrange("(c p) f -> p c f", p=128),
        )
        WTs = consts.tile([64, KC, 128], F32)
        for rc in range(KC):
            wtps = psM.tile([64, 128], F32, tag="m")
            nc.tensor.transpose(wtps, Wf[:, rc, 0:F_OUT], idf)
            nc.any.tensor_copy(WTs[:, rc, :], wtps)
        ws8f = consts.tile([128, KC], F32)
        for rc in range(KC):
            wps = psM.tile([128, 2], F32, tag="m")
            nc.tensor.matmul(wps, lhsT=WTs[:, rc, :], rhs=a2s, start=True, stop=True)
            nc.any.tensor_copy(ws8f[:, rc : rc + 1], wps[:, 0:1])
            nc.any.tensor_copy(Wf[:, rc, F_OUT : F_OUT + 1], wps[:, 1:2])
        # bf16 versions for the bf16 matmuls
        Waug = consts.tile([128, KC, F_OUT + 1], BF16)
        nc.vector.tensor_copy(Waug, Wf)
        wsrc08 = consts.tile([128, KC], BF16)
        nc.vector.tensor_copy(wsrc08, ws8f)

        # ---------------- own-rows f_src08 -> broadcast row ----------------
        hTo = consts.tile([128, KC, ROWS], BF16)
        nc.gpsimd.dma_start(
            out=hTo, in_=hTs[:, :].rearrange("(c p) n -> p c n", p=128)
        )
        fps = psM.tile([128, LCH], F32, tag="m")
        for ic in range(LCH):
            for kc in range(KC):
                nc.tensor.matmul(
                    fps[:, ic : ic + 1],
                    lhsT=hTo[:, kc, ic * 128 : (ic + 1) * 128],
                    rhs=wsrc08[:, kc : kc + 1],
                    start=(kc == 0),
                    stop=(kc == KC - 1),
                )
        fso = consts.tile([128, LCH], F32)
        nc.any.tensor_copy(fso, fps)
        fsTps = psM.tile([LCH, 128], F32, tag="m")
        nc.tensor.transpose(fsTps, fso, idf)
        fsTs = consts.tile([LCH, 128], BF16)
        nc.any.tensor_copy(fsTs, fsTps)
        nc.gpsimd.dma_start(out=fsd[:].rearrange("(q p) -> q p", p=128), in_=fsTs)
        fs08row = consts.tile([128, ROWS], BF16)
        fsd_bc = bass.AP(tensor=fsd, offset=0, ap=[[0, 128], [1, ROWS]])
        nc.gpsimd.dma_start(out=fs08row, in_=fsd_bc)
        # es = exp(0.8 f_src) broadcast row (for factorized windows)
        es_row = consts.tile([128, ROWS], BF16)
        nc.scalar.activation(es_row, fs08row, Act.Exp, bias=0.0, scale=1.0)

        # ---------------- persistent attention-side tiles ----------------
        bmat = consts.tile([128, MCH], F32)  # exp(0.2 f_dst)
        fd08 = consts.tile([128, MCH], F32)  # 0.8 f_dst (ACT bias)
        edcol = consts.tile([128, MCH], F32)  # exp(0.8 f_dst)
        rhs_aug = consts.tile([128, MCH, F_OUT + 1], BF16)

        acc0 = psAcc.tile([F_OUT + 1, 512], F32, tag="a0")
        acc1 = psAcc.tile([F_OUT + 1, 512], F32, tag="a1")

        # ---------------- pre-phase: Wh|f_dst per half-group ----------------
        for hg in range(NHG):
            if hg % 2 == 0:
                g = hg // 2
                gsl = slice(g * 1024, (g + 1) * 1024)
                hTg = hp.tile([128, KC, 1024], BF16, tag="hTg")
                nc.gpsimd.dma_start(
                    out=hTg, in_=hT[:, gsl].rearrange("(c p) n -> p c n", p=128)
                )
            off = (hg % 2) * HG * 128  # column offset inside hTg
            hs = slice(hg * HG, (hg + 1) * HG)
            whH = psW.tile([128, HG, F_OUT + 1], F32, tag="wh")
            for q in range(HG):
                for kc in range(KC):
                    nc.tensor.matmul(
                        whH[:, q, :],
                        lhsT=hTg[:, kc, off + q * 128 : off + (q + 1) * 128],
                        rhs=Waug[:, kc, :],
                        start=(kc == 0),
                        stop=(kc == KC - 1),
                    )
            fdv = whH[:, :, F_OUT]  # [128, HG] strided view
            nc.scalar.activation(bmat[:, hs], fdv, Act.Exp, bias=0.0, scale=0.2)
            nc.scalar.activation(edcol[:, hs], fdv, Act.Exp, bias=0.0, scale=0.8)
            nc.vector.tensor_scalar(fd08[:, hs], fdv, 0.8, None, Alu.mult)
            # rhs_aug[:, hs, 0:64] = whH * bmat (stride-0 broadcast of bmat)
            bm = bmat[:, hs]
            bmb = bass.AP(
                tensor=bm.tensor,
                offset=bm.offset,
                ap=[list(bm.ap[0]), list(bm.ap[1]), [0, F_OUT]],
            )
            nc.vector.tensor_tensor(
                rhs_aug[:, hs, 0:F_OUT], whH[:, :, 0:F_OUT], bmb, Alu.mult
            )
            nc.vector.tensor_copy(rhs_aug[:, hs, F_OUT], bmat[:, hs])

        # ---------------- main loop ----------------
        deferred = {}
        for mc in range(MCH):
            adjm = adjp.tile([128, ROWS], BF16, tag="adj")
            nc.sync.dma_start(out=adjm, in_=adjT[mc * 128 : (mc + 1) * 128, :])
            Xm = mk.tile([128, ROWS], BF16, tag="Xm")
            if mc in facset:
                nc.vector.tensor_scalar(
                    Xm, es_row, edcol[:, mc : mc + 1], 1.0, Alu.mult, Alu.max
                )
            else:
                X = xk.tile([128, ROWS], BF16, tag="X")
                nc.scalar.activation(
                    X, fs08row, Act.Exp, bias=fd08[:, mc : mc + 1], scale=1.0
                )
                nc.vector.tensor_scalar(Xm, X, 1.0, None, Alu.max)
            if mc in poolset:
                # gpsimd mask-mult; matmul deferred so the PE stream
                # never waits on the slower engine
                p = pdef.tile([128, ROWS], BF16, tag="pd")
                nc.gpsimd.tensor_tensor(p, Xm, adjm, Alu.mult)
                deferred[mc] = p
            else:
                p = pk.tile([128, ROWS], BF16, tag="p")
                nc.vector.tensor_tensor(p, Xm, adjm, Alu.mult)
                nc.tensor.matmul(
                    acc0, lhsT=rhs_aug[:, mc, :], rhs=p[:, 0:512],
                    start=(mc == 0), stop=False,
                )
                nc.tensor.matmul(
                    acc1, lhsT=rhs_aug[:, mc, :], rhs=p[:, 512:1024],
                    start=(mc == 0), stop=False,
                )
        for i, mc in enumerate(mm_tail):
            p = deferred[mc]
            last = i == len(mm_tail) - 1
            nc.tensor.matmul(
                acc0, lhsT=rhs_aug[:, mc, :], rhs=p[:, 0:512],
                start=False, stop=last,
            )
            nc.tensor.matmul(
                acc1, lhsT=rhs_aug[:, mc, :], rhs=p[:, 512:1024],
                start=False, stop=last,
            )

        # ---------------- epilogue: transpose accT, divide, elu ----------------
        accS = ep.tile([F_OUT + 1, ROWS], F32, tag="accS")
        nc.any.tensor_copy(accS[:, 0:512], acc0)
        nc.any.tensor_copy(accS[:, 512:1024], acc1)
        scS = ep.tile([128, LCH, F_OUT + 1], F32, tag="scS")
        for q in range(LCH):
            trp = psM.tile([128, F_OUT + 1], F32, tag="m")
            nc.tensor.transpose(
                trp,
                accS[:, q * 128 : (q + 1) * 128],
                idf[0 : F_OUT + 1, 0 : F_OUT + 1],
            )
            nc.any.tensor_copy(scS[:, q, :], trp)
        rzS = ep.tile([128, LCH], F32, tag="rzS")
        nc.vector.reciprocal(rzS, scS[:, :, F_OUT])
        rzb = bass.AP(
            tensor=rzS.tensor,
            offset=rzS.offset,
            ap=[list(rzS.ap[0]), list(rzS.ap[1]), [0, F_OUT]],
        )
        hpS = ep.tile([128, LCH, F_OUT], F32, tag="hpS")
        nc.vector.tensor_tensor(hpS, scS[:, :, 0:F_OUT], rzb, Alu.mult)
        # elu(x) = max(x,0) + exp(min(x,0)) - 1
        mnS = ep.tile([128, LCH, F_OUT], F32, tag="mnS")
        nc.vector.tensor_scalar(mnS, hpS, 0.0, None, Alu.min)
        emS = ep.tile([128, LCH, F_OUT], F32, tag="emS")
        nc.scalar.activation(emS, mnS, Act.Exp, bias=0.0, scale=1.0)
        rpS = ep.tile([128, LCH, F_OUT], F32, tag="rpS")
        nc.vector.tensor_scalar(rpS, hpS, 0.0, None, Alu.max)
        s1S = ep.tile([128, LCH, F_OUT], F32, tag="s1S")
        nc.vector.tensor_tensor(s1S, emS, rpS, Alu.add)
        obS = ep.tile([128, LCH, F_OUT], F32, tag="obS")
        nc.vector.tensor_scalar(obS, s1S, -1.0, None, Alu.add)
        nc.gpsimd.dma_start(
            out=out[:, :].rearrange("(q p) f -> p q f", p=128), in_=obS
        )


def _get_nc(repeat=1):
    import os

    key = (
        "nc",
        repeat,
        os.environ.get("GAT_ADJBUFS", ""),
        os.environ.get("GAT_FACWIN", ""),
        os.environ.get("GAT_POOLM", ""),
    )
    if key not in _CACHE:
        _CACHE[key] = _build_nc(repeat)
    return _CACHE[key]


def _make_in_maps(h, adj, W, a):
    import ml_dtypes

    bf16 = ml_dtypes.bfloat16
    h = np.ascontiguousarray(h, dtype=np.float32)
    adj = np.ascontiguousarray(adj, dtype=np.int32)
    W = np.ascontiguousarray(W, dtype=np.float32)
    a = np.ascontiguousarray(a, dtype=np.float32)
    hT16 = h.T.astype(bf16)  # [256, 8192] bf16, contiguous
    in_maps = []
    for c in range(N_CORES):
        sl = slice(c * ROWS, (c + 1) * ROWS)
        adjTc = np.ascontiguousarray(adj[sl].T)  # [8192, 1024] int32
        in_maps.append(
            {
                "hT": hT16,
                "hTs": np.ascontiguousarray(hT16[:, sl]),
                "adjT": adjTc.astype(bf16),  # 0/1 values, exact in bf16
                "W": W,
                "a": a,
            }
        )
    return in_maps


def kernel(h, adj, W, a, _collect_results=False, _trace=False):
    in_maps = _make_in_maps(h, adj, W, a)
    nc = _get_nc()
    res = run_bass_kernel_spmd(nc, in_maps, list(range(N_CORES)), trace=_trace)
    out = np.concatenate([res.results[c]["out"] for c in range(N_CORES)], axis=0)
    out = np.ascontiguousarray(out, dtype=np.float32)
    if _collect_results:
        return out, res
    return out
